# revision 1
# baseline (speedup 1.0000x reference)
"""BCQLinear packed forward on 8 Trainium2 NeuronCores.

Column-parallel (tensor-parallel) sharding: binary/alpha/bias are sharded
along out_features (dim 0, 4096 -> 8 x 512); the input activations are
replicated. Each core dequantizes its weight shard
    W[o, i] = sum_b alpha[o, g, b] * binary[o, g, a, b],   i = 128 g + a
on the vector engine, transposes it through the PE array to Wt[i, o], and
runs a K-contiguous fp32r matmul  out[ms, o] = x[ms, :] @ Wt[:, o] + bias.
The host concatenates the 8 output shards along o.

Shapes are hardcoded for the problem instance:
  input  [2, 1024, 4096] f32
  binary [4096, 32, 128, 3] f32 (+-1)
  alpha  [4096, 32, 3] f32
  bias   [4096] f32
"""

import numpy as np
from contextlib import ExitStack

import bass_rust
import concourse.bass as bass
import concourse.mybir as mybir
import concourse.tile as tile
from concourse.bass_utils import run_bass_kernel_spmd
from concourse.masks import make_identity


def _legalize_waits(nc, max_waits=1):
    """Walrus codegen allows only one sync-wait on (at least) DVE
    TensorTensor instructions. Move excess waits onto injected same-engine
    NoOps placed immediately before the instruction (program order per
    engine preserves the semantics)."""
    seq = 0
    for fn in nc.m.functions:
        for blk in fn.blocks:
            new_insts = []
            changed = False
            for inst in blk.instructions:
                si = inst.sync_info
                if si is not None and len(si.on_wait) > max_waits:
                    waits = list(si.on_wait)
                    for w in waits[:-max_waits]:
                        nop = mybir.InstNoOp(name=f"wlegal-{seq}")
                        seq += 1
                        nop.engine = inst.engine
                        nop.sync_info = bass_rust.SyncInfo(
                            on_wait=[w], on_update=[])
                        new_insts.append(nop)
                    inst.sync_info = bass_rust.SyncInfo(
                        on_wait=waits[-max_waits:],
                        on_update=list(si.on_update))
                    changed = True
                new_insts.append(inst)
            if changed:
                blk.instructions = new_insts

P = 128          # partitions
N_CORES = 8
B, S = 2, 1024
MS = B * S       # 2048 tokens
I = 4096         # in_features
O = 4096         # out_features
O_SH = O // N_CORES  # 512 per core
G, A, NB = 32, 128, 3
KT = I // P      # 32 contraction tiles
MB = MS // P     # 16 token blocks
OT = O_SH // P   # 4 o-tiles per core

F32 = mybir.dt.float32
F32R = mybir.dt.float32r
BIN_DTYPES = {
    "f32": mybir.dt.float32,
    "bf16": mybir.dt.bfloat16,
    "fp8": mybir.dt.float8e4,
}

_CACHED = {}


def build_nc(mm_f32r: bool = True, bin_dtype: str = "fp8",
             fuse_transpose: bool = True, x_bufs: int = 3,
             pool_ots: tuple = (), pool_planes: int = 0,
             repeat: int = 1, plane_bufs: int = 7,
             b_bufs: int = 5, plane_split: int = 2) -> bass.Bass:
    nc = bass.Bass("TRN2", target_bir_lowering=False, debug=False)
    MMDT = F32R if mm_f32r else F32
    BDT = BIN_DTYPES[bin_dtype]

    # Host-staged layouts (pure relayouts of the sharded inputs):
    #  xb    [MB, P, KT, P]  : xb[m, p, k, j] = x[m*128+j, k*128+p]
    #  bperm [O_SH, NB, G, A]: binary shard with the bit axis moved outward
    #  alpha [O_SH, G, NB]
    #  biasr [P, O_SH]       : bias shard replicated across partitions
    xb_d = nc.dram_tensor("xb", [MB, P, KT, P], MMDT, kind="ExternalInput").ap()
    b_d = nc.dram_tensor("bperm", [O_SH, NB, G, A], BDT, kind="ExternalInput").ap()
    al_d = nc.dram_tensor("alpha", [O_SH, G, NB], F32, kind="ExternalInput").ap()
    bias_d = nc.dram_tensor("biasr", [P, O_SH], F32, kind="ExternalInput").ap()
    out_d = nc.dram_tensor("out", [MS, O_SH], F32, kind="ExternalOutput").ap()
    out_t = out_d.rearrange("(mb p) o -> mb p o", p=P)

    mult = mybir.AluOpType.mult
    add = mybir.AluOpType.add

    with tile.TileContext(nc) as tc, ExitStack() as ctx:
        const = ctx.enter_context(tc.tile_pool(name="const", bufs=1))
        wt_pool = ctx.enter_context(tc.tile_pool(name="wt", bufs=1))
        bpool = ctx.enter_context(tc.tile_pool(name="bin", bufs=b_bufs))
        wpool = ctx.enter_context(
            tc.tile_pool(name="w", bufs=plane_bufs if fuse_transpose else 2))
        xpool = ctx.enter_context(tc.tile_pool(name="x", bufs=x_bufs))
        opool = ctx.enter_context(tc.tile_pool(name="o", bufs=2))
        ps_mm = ctx.enter_context(tc.tile_pool(name="psmm", bufs=2, space="PSUM"))
        ps_tr = ctx.enter_context(tc.tile_pool(name="pstr", bufs=4, space="PSUM"))

        ident = const.tile([P, P], F32)
        make_identity(nc, ident)
        if mm_f32r:
            # is_transpose matmuls on f32r planes need an f32r identity and
            # an explicitly-rounded producer (DVE copy rounds to f32r).
            ident_mm = const.tile([P, P], F32R, tag="identr")
            nc.vector.tensor_copy(ident_mm, ident)
        else:
            ident_mm = ident
        bias_f32 = const.tile([P, O_SH], F32)
        nc.sync.dma_start(bias_f32, bias_d)
        al_sb = const.tile([P, OT, G * NB], F32)
        nc.sync.dma_start(al_sb, al_d.rearrange("(ot p) g nb -> p ot (g nb)", p=P))

        # ---- Dequant + transpose, one o-tile (128 out channels) at a time
        al3 = al_sb.rearrange("p ot (g nb) -> p ot g nb", nb=NB)

        def emit_dequant(wt_sb, ot):
            def al_bc(b, ot=ot):
                # alpha[o_p, g, b] broadcast along a: [P, G, 1] -> [P, G, A]
                return al3[:, ot, :, b:b + 1].to_broadcast([P, G, A])

            eng = nc.gpsimd if ot in pool_ots else nc.vector
            if fuse_transpose:
                # Scale each +-1 bit-plane by its alpha (DVE), then let the
                # PE transpose-accumulate the three planes in PSUM:
                # Wt[a, o] = sum_b (alpha_b * B_b)[o, a].
                # Planes are emitted in half-G granularity so DVE scaling,
                # PE transposes and ACT copies pipeline across o-tiles.
                GH = G // plane_split
                b_tiles = []
                for b in range(NB):
                    b_sb = bpool.tile([P, G, A], BDT)
                    nc.sync.dma_start(b_sb, b_d[ot * P:(ot + 1) * P, b])
                    b_tiles.append(b_sb)
                for gh in range(plane_split):
                    gsl = slice(gh * GH, (gh + 1) * GH)
                    planes = []
                    for b in range(NB):
                        pl = wpool.tile([P, GH, A], MMDT, tag="plane")
                        e = nc.gpsimd if b >= NB - pool_planes else eng
                        e.tensor_tensor(pl, b_tiles[b][:, gsl],
                                        al3[:, ot, gsl, b:b + 1]
                                        .to_broadcast([P, GH, A]), mult)
                        planes.append(pl)
                    for gg in range(GH):
                        g = gh * GH + gg
                        ps = ps_tr.tile([P, P], MMDT)
                        for b in range(NB):
                            nc.tensor.matmul(ps, planes[b][:, gg], ident_mm,
                                             is_transpose=True,
                                             start=(b == 0), stop=(b == NB - 1))
                        nc.scalar.copy(wt_sb[:, g, ot * P:(ot + 1) * P], ps)
            else:
                b_tiles = []
                for b in range(NB):
                    b_sb = bpool.tile([P, G, A], BDT)
                    nc.sync.dma_start(b_sb, b_d[ot * P:(ot + 1) * P, b])
                    b_tiles.append(b_sb)
                w_sb = wpool.tile([P, G, A], F32)  # W[o_p, g, a]
                if BDT == F32:
                    eng.tensor_tensor(b_tiles[1], b_tiles[1], al_bc(1), mult)
                    eng.tensor_tensor(b_tiles[2], b_tiles[2], al_bc(2), mult)
                    eng.tensor_tensor(w_sb, b_tiles[0], al_bc(0), mult)
                    eng.tensor_tensor(w_sb, w_sb, b_tiles[1], add)
                    eng.tensor_tensor(w_sb, w_sb, b_tiles[2], add)
                else:
                    t_sb = wpool.tile([P, G, A], F32, tag="tmp")
                    eng.tensor_tensor(w_sb, b_tiles[0], al_bc(0), mult)
                    eng.tensor_tensor(t_sb, b_tiles[1], al_bc(1), mult)
                    eng.tensor_tensor(w_sb, w_sb, t_sb, add)
                    eng.tensor_tensor(t_sb, b_tiles[2], al_bc(2), mult)
                    eng.tensor_tensor(w_sb, w_sb, t_sb, add)
                for g in range(G):
                    ps = ps_tr.tile([P, P], F32)
                    nc.tensor.transpose(ps, w_sb[:, g], ident)
                    nc.scalar.copy(wt_sb[:, g, ot * P:(ot + 1) * P], ps)

        def emit_mm(wt_sb, m):
            # Matmul for one 128-token block: contract all 32 k-tiles
            xt_sb = xpool.tile([P, KT, P], MMDT)
            nc.sync.dma_start(xt_sb, xb_d[m])
            ps = ps_mm.tile([P, O_SH], F32)
            for k in range(KT):
                nc.tensor.matmul(ps, xt_sb[:, k], wt_sb[:, k],
                                 start=(k == 0), stop=(k == KT - 1))
            out_sb = opool.tile([P, O_SH], F32)
            nc.vector.tensor_tensor(out_sb, ps, bias_f32, add)
            nc.sync.dma_start(out_t[m], out_sb)

        for _rep in range(repeat):
            # Wt[i, o] resident: [P(i%128), KT, O_SH]
            wt_sb = wt_pool.tile([P, KT, O_SH], MMDT, tag="wt")
            for ot in range(OT):
                emit_dequant(wt_sb, ot)
            for m in range(MB):
                emit_mm(wt_sb, m)

    _legalize_waits(nc)
    return nc


def _stage_inputs(input, binary, alpha, bias, bin_dtype="fp8"):
    x = np.ascontiguousarray(np.asarray(input, dtype=np.float32)).reshape(MS, I)
    # xb[m, p, k, j] = x[m*128+j, k*128+p]
    xb = np.ascontiguousarray(
        x.reshape(MB, P, KT, P).transpose(0, 3, 2, 1))
    # binary is strictly +-1, exactly representable in bf16/fp8e4 — the cast
    # during staging is lossless.
    np_bdt = mybir.dt.np(BIN_DTYPES[bin_dtype])
    bperm = np.ascontiguousarray(
        np.asarray(binary, dtype=np.float32).transpose(0, 3, 1, 2)
    ).astype(np_bdt)
    alpha = np.ascontiguousarray(np.asarray(alpha, dtype=np.float32))
    bias = np.asarray(bias, dtype=np.float32)

    in_maps = []
    for c in range(N_CORES):
        sl = slice(c * O_SH, (c + 1) * O_SH)
        in_maps.append({
            "xb": xb,
            "bperm": np.ascontiguousarray(bperm[sl]),
            "alpha": np.ascontiguousarray(alpha[sl]),
            "biasr": np.ascontiguousarray(
                np.broadcast_to(bias[sl][None, :], (P, O_SH))),
        })
    return in_maps


def kernel(input, binary, alpha, bias, _trace=False, _mm_f32r=True,
           _bin_dtype="fp8", _fuse_transpose=True, _x_bufs=3, _pool_ots=(),
           _pool_planes=0):
    key = (_mm_f32r, _bin_dtype, _fuse_transpose, _x_bufs, tuple(_pool_ots),
           _pool_planes)
    if key not in _CACHED:
        _CACHED[key] = build_nc(mm_f32r=_mm_f32r, bin_dtype=_bin_dtype,
                                fuse_transpose=_fuse_transpose,
                                x_bufs=_x_bufs, pool_ots=tuple(_pool_ots),
                                pool_planes=_pool_planes)
    nc = _CACHED[key]
    in_maps = _stage_inputs(input, binary, alpha, bias, bin_dtype=_bin_dtype)
    res = run_bass_kernel_spmd(nc, in_maps, core_ids=list(range(N_CORES)),
                               trace=_trace)
    out = np.concatenate([res.results[c]["out"] for c in range(N_CORES)],
                         axis=1).reshape(B, S, O).astype(np.float32)
    if _trace:
        kernel.last_result = res
    return out

